# revision 6
# baseline (speedup 1.0000x reference)
"""Trainium2 Bass kernel for BlockIndexNet (per-species MLP over atom blocks).

One species block per core (8 cores, data-parallel). Host gathers +
transposes each species' embeddings to [D_IN, 25000] f16; device runs the
3-layer MLP with activations kept transposed ([feature, atom]); matmuls in
f16 (1 col/cycle at 2.4 GHz). PE and ACT engines are co-paced at their
rooflines (~2.16us/tile each).

Design:
- Manual PSUM bank plan, one tensor [128, g, par, hh, 512] = 8 banks:
  g=0 (banks 0-3) holds L1 matmul outputs double-buffered by tile parity
  `par`; g=1 (banks 4-7) holds L2 outputs the same way; L3's matmul reuses
  the L2 bank its silu just drained, and the DVE casts it to f16 SBUF for
  the output DMA. Double-buffering both layers removes the mm1->silu1
  serialization an earlier version had.
- Software-pipeline skew per iteration t:
  PE [mm1(t+1), mm2(t), mm3(t-1)], ACT [silu2(t-1), silu1(t+1)],
  DVE [cast(t-1)] - each engine consumes data produced a full period
  earlier, so neither PE nor ACT waits on the other's freshest output and
  the PE stays dense (PE idle gaps invite a ~1.2x whole-core down-clock).
- No padding: 48 tiles of 512 atoms + one 424-atom tail tile cover the
  25000-atom block exactly.
- Output written f16 (halves output DMA traffic); host upcasts.
- 6 warm matmuls bridge the initial input-DMA latency; tile-0's input and
  w1 are the first DMAs so mm1(0) starts as early as possible.
"""

import sys

if "/opt/trn_rl_repo" not in sys.path:
    sys.path.insert(0, "/opt/trn_rl_repo")

import numpy as np

N_ATOMS = 200_000
D_IN = 256
H = 256
D_OUT = 128
N_SPECIES = 8
BLOCK = N_ATOMS // N_SPECIES  # 25000

B_TILE = 512
N_TILES = 49
B_PAD = BLOCK  # 25000 = 48*512 + 424; the last tile is short (no padding)
LAST_N = BLOCK - (N_TILES - 1) * B_TILE  # 424

_P = 128
_KC = D_IN // _P  # 2 contraction chunks per layer

WEIGHT_DT = "f16"
ACT_DT = "f16"
OUT_DT = "f16"
WARM_MMS = 6

_program_cache: dict = {}


def _np_dtype(name):
    if name == "bf16":
        import ml_dtypes

        return ml_dtypes.bfloat16
    if name == "f16":
        return np.float16
    return np.float32


def _build_program(zero_bias: bool):
    import concourse.bacc as bacc
    import concourse.mybir as mybir
    from concourse.tile import TileContext

    f32 = mybir.dt.float32
    _dtmap = {"f32r": mybir.dt.float32r, "bf16": mybir.dt.bfloat16,
              "f16": mybir.dt.float16}
    w_dt = _dtmap[WEIGHT_DT]
    a_dt = _dtmap[ACT_DT]
    o_dt = _dtmap[OUT_DT]
    SILU = mybir.ActivationFunctionType.Silu

    nc = bacc.Bacc("TRN2", num_devices=N_SPECIES)

    xt_d = nc.dram_tensor("xt", [D_IN, B_PAD], a_dt, kind="ExternalInput")
    w1_d = nc.dram_tensor("w1", [D_IN, H], w_dt, kind="ExternalInput")
    w2_d = nc.dram_tensor("w2", [H, H], w_dt, kind="ExternalInput")
    w3_d = nc.dram_tensor("w3", [H, D_OUT], w_dt, kind="ExternalInput")
    if not zero_bias:
        b1_d = nc.dram_tensor("b1", [H], f32, kind="ExternalInput")
        b2_d = nc.dram_tensor("b2", [H], f32, kind="ExternalInput")
        b3_d = nc.dram_tensor("b3", [D_OUT], f32, kind="ExternalInput")
    yt_d = nc.dram_tensor("yt", [D_OUT, B_PAD], o_dt, kind="ExternalOutput")

    xt_v = xt_d.rearrange("(kc p) n -> p kc n", p=_P)

    with TileContext(nc) as tc:
        with (
            nc.psum_tensor("ps", [_P, 2, 2, 2, B_TILE], f32) as ps,
            tc.tile_pool(name="wpool", bufs=1) as wpool,
            tc.tile_pool(name="xpool", bufs=5) as xpool,
            tc.tile_pool(name="h1pool", bufs=4) as h1pool,
            tc.tile_pool(name="h2pool", bufs=4) as h2pool,
            tc.tile_pool(name="opool", bufs=4) as opool,
        ):
            # ps bank index = g*4 + par*2 + hh:
            #   [:, 0, t%2, hh, :]  L1 (mm1) output for tile t
            #   [:, 1, t%2, hh, :]  L2 (mm2) output for tile t
            #   [:, 1, t%2, 1, :]   L3 (mm3) output for tile t (reuses the
            #                       hh=1 bank silu2(t) just drained, so
            #                       mm2(t+2)'s first hh=0 matmuls don't wait
            #                       on the output cast)

            # Warm-up: preload the SILU table and keep the PE busy through
            # the input-DMA latency so the HAM clock gate is at full rate
            # when real matmuls arrive.
            warm_sb = wpool.tile([_P, B_TILE], a_dt, tag="warm")
            warm_out = wpool.tile([_P, 16], a_dt, tag="warm_out")
            nc.vector.memset(warm_sb[:], 0.0)
            nc.scalar.activation(warm_out[:], warm_sb[:, :16], SILU)

            w1_sb = wpool.tile([_P, _KC, H], w_dt, tag="w1")
            w2_sb = wpool.tile([_P, _KC, H], w_dt, tag="w2")
            w3_sb = wpool.tile([_P, _KC, D_OUT], w_dt, tag="w3")
            if not zero_bias:
                b1_sb = wpool.tile([_P, 2], f32, tag="b1")
                b2_sb = wpool.tile([_P, 2], f32, tag="b2")
                b3_sb = wpool.tile([_P, 1], f32, tag="b3")

            w_loads = [
                (w1_sb, w1_d), (w2_sb, w2_d), (w3_sb, w3_d),
            ]
            if not zero_bias:
                nc.sync.dma_start(b1_sb[:], b1_d.rearrange("(hh p) -> p hh", p=_P))
                nc.sync.dma_start(b2_sb[:], b2_d.rearrange("(hh p) -> p hh", p=_P))
                nc.sync.dma_start(b3_sb[:], b3_d.rearrange("(hh p) -> p hh", p=_P))

            xts = {}
            h1s = {}
            h2s = {}

            def dma_x(t):
                # pair-granular loads (1 MiB); pair 0 is loaded half-at-a-time
                # in the prologue so mm1(0) can start as soon as tile 0 lands.
                if t % 2 == 1 or t == 0:
                    return
                g = t // 2
                n = min(2 * B_TILE, B_PAD - t * B_TILE)
                xts[g] = xpool.tile([_P, _KC, 2 * B_TILE], a_dt, tag="x",
                                    name=f"x_{g}")
                nc.sync.dma_start(xts[g][:, :, :n],
                                  xt_v[:, :, t * B_TILE:t * B_TILE + n])

            for _ in range(WARM_MMS):
                nc.tensor.matmul(ps[:, 1, 1, 1, :], warm_sb[:, :_P],
                                 warm_sb[:], start=True, stop=True)

            def nt(t):
                return LAST_N if t == N_TILES - 1 else B_TILE

            def mm1(t):
                g, c, n = t // 2, t % 2, nt(t)
                for hh in range(2):
                    for kc in range(_KC):
                        nc.tensor.matmul(
                            ps[:, 0, t % 2, hh, :n],
                            w1_sb[:, kc, hh * _P:(hh + 1) * _P],
                            xts[g][:, kc, c * B_TILE:c * B_TILE + n],
                            start=(kc == 0),
                            stop=(kc == _KC - 1),
                        )
                if c == 1 or t == N_TILES - 1:
                    del xts[g]

            def mm2(t):
                src, n = h1s[t], nt(t)
                for hh in range(2):
                    for kc in range(_KC):
                        nc.tensor.matmul(
                            ps[:, 1, t % 2, hh, :n],
                            w2_sb[:, kc, hh * _P:(hh + 1) * _P],
                            src[:, kc, :n],
                            start=(kc == 0),
                            stop=(kc == _KC - 1),
                        )
                del h1s[t]

            def mm3(t):
                # L3 lands in the hh=1 bank so mm2(t+2)'s first (hh=0)
                # matmuls don't sit behind the output cast's WAR.
                src, n = h2s[t], nt(t)
                for kc in range(_KC):
                    nc.tensor.matmul(
                        ps[:, 1, t % 2, 1, :n],
                        w3_sb[:, kc, :],
                        src[:, kc, :n],
                        start=(kc == 0),
                        stop=(kc == _KC - 1),
                    )
                del h2s[t]

            def silu1(t):
                n = nt(t)
                tile = h1pool.tile([_P, _KC, B_TILE], a_dt, tag="h1",
                                   name=f"h1_{t}")
                h1s[t] = tile
                if zero_bias:
                    nc.scalar.activation(tile[:, :, :n],
                                         ps[:, 0, t % 2, :, :n], SILU)
                else:
                    for hh in range(2):
                        nc.scalar.activation(
                            tile[:, hh, :n], ps[:, 0, t % 2, hh, :n], SILU,
                            bias=b1_sb[:, hh:hh + 1])

            def silu2(t):
                n = nt(t)
                tile = h2pool.tile([_P, _KC, B_TILE], a_dt, tag="h2",
                                   name=f"h2_{t}")
                h2s[t] = tile
                if zero_bias:
                    nc.scalar.activation(tile[:, :, :n],
                                         ps[:, 1, t % 2, :, :n], SILU)
                else:
                    for hh in range(2):
                        nc.scalar.activation(
                            tile[:, hh, :n], ps[:, 1, t % 2, hh, :n], SILU,
                            bias=b2_sb[:, hh:hh + 1])

            def tail(t):
                n = nt(t)
                out_sb = opool.tile([_P, B_TILE], o_dt, tag="o", name=f"o_{t}")
                if zero_bias:
                    nc.vector.tensor_copy(out_sb[:, :n], ps[:, 1, t % 2, 1, :n])
                else:
                    nc.vector.tensor_scalar_add(out_sb[:, :n],
                                                ps[:, 1, t % 2, 1, :n],
                                                b3_sb[:, 0:1])
                nc.sync.dma_start(yt_d[:, t * B_TILE:t * B_TILE + n],
                                  out_sb[:, :n])

            # prologue: tile 0's input first (mm1(0)'s critical path), then
            # w1, then tile 1 + the other weights, then the prefetch window.
            xts[0] = xpool.tile([_P, _KC, 2 * B_TILE], a_dt, tag="x", name="x_0")
            nc.sync.dma_start(xts[0][:, :, :B_TILE], xt_v[:, :, :B_TILE])
            nc.sync.dma_start(w_loads[0][0][:],
                              w_loads[0][1].rearrange("(kc p) m -> p kc m", p=_P))
            nc.sync.dma_start(xts[0][:, :, B_TILE:2 * B_TILE],
                              xt_v[:, :, B_TILE:2 * B_TILE])
            for w_sb, w_d in w_loads[1:]:
                nc.sync.dma_start(w_sb[:], w_d.rearrange("(kc p) m -> p kc m", p=_P))
            for t in (2, 4):
                dma_x(t)
            mm1(0)
            silu1(0)
            mm1(1)
            silu1(1)
            mm2(0)
            # steady state: iteration t. ACT FIFO per iter is
            # [silu2(t-1), silu1(t+1)]; PE FIFO [mm1(t+1), mm2(t), mm3(t-1)].
            # The skew keeps each engine one producer ahead of its consumer.
            for t in range(1, N_TILES):
                if t + 5 < N_TILES:
                    dma_x(t + 5)
                if t + 1 < N_TILES:
                    mm1(t + 1)
                silu2(t - 1)
                if t + 1 < N_TILES:
                    silu1(t + 1)
                mm2(t)
                if t >= 1:
                    mm3(t - 1)
                    tail(t - 1)
            silu2(N_TILES - 1)
            mm3(N_TILES - 1)
            tail(N_TILES - 1)

    nc.compile()
    return nc


def _get_program(zero_bias: bool):
    key = ("prog", zero_bias)
    if key not in _program_cache:
        _program_cache[key] = _build_program(zero_bias)
    return _program_cache[key]


def run(embedding, W1, b1, W2, b2, W3, b3, species, block_index, trace=False,
        trace_cores=None):
    """Core implementation; returns (full_output, BassKernelResults)."""
    from concourse.bass_utils import run_bass_kernel_spmd

    embedding = np.ascontiguousarray(np.asarray(embedding, dtype=np.float32))
    W1 = np.asarray(W1, dtype=np.float32)
    b1 = np.asarray(b1, dtype=np.float32)
    W2 = np.asarray(W2, dtype=np.float32)
    b2 = np.asarray(b2, dtype=np.float32)
    W3 = np.asarray(W3, dtype=np.float32)
    b3 = np.asarray(b3, dtype=np.float32)
    block_index = np.asarray(block_index)

    zero_bias = not (b1.any() or b2.any() or b3.any())
    nc = _get_program(zero_bias)
    wdt = _np_dtype(WEIGHT_DT)
    adt = _np_dtype(ACT_DT)

    gathered = embedding[block_index.reshape(-1)].reshape(N_SPECIES, BLOCK, D_IN)
    in_maps = []
    for s in range(N_SPECIES):
        xt = np.ascontiguousarray(gathered[s].T, dtype=adt)
        m = {"xt": xt, "w1": W1[s].astype(wdt), "w2": W2[s].astype(wdt),
             "w3": W3[s].astype(wdt)}
        if not zero_bias:
            m["b1"] = b1[s]
            m["b2"] = b2[s]
            m["b3"] = b3[s]
        in_maps.append(m)

    res = run_bass_kernel_spmd(
        nc, in_maps, core_ids=list(range(N_SPECIES)), trace=trace,
        trace_cores=trace_cores,
    )

    n_out = np.asarray(species).shape[0]
    out = np.zeros((n_out, D_OUT), dtype=np.float32)
    for s in range(N_SPECIES):
        out[block_index[s]] = res.results[s]["yt"][:, :BLOCK].T.astype(np.float32)
    return out, res


def kernel(**inputs) -> np.ndarray:
    out, _ = run(**inputs)
    return out
